# Initial kernel scaffold
#
"""GAT head (DGAT) Trainium2 kernel: 8-core row-sharded masked-softmax attention.

Math (per reference):
  h = X @ W                       [N, 64]
  e = leaky_relu(src_i + dst_j, 0.2), src = h@a[:64], dst = h@a[64:]
  att = softmax(where(adj>0, e, -9e15), axis=1)
  out = elu(att @ h)

Kernel strategy per core c (rows R = [1024c, 1024c+1024)):
  - DMA-cast adj slab int32 -> bf16 {0,1} (SWDGE cast during DMA)
  - PE: u[j, i] = BIG*adjT (transpose-matmul vs BIG*I, bf16)
                + src_i     (K=1 fp32 rank-1 matmul, ones x src row)
  - ACT: e = prelu(u + (dst_j - BIG), alpha=0.2)   [masked: ~0.2*(s-BIG) -> exp -> 0]
         p = exp(e)
  - PE: agg[65, i] += hext_jc^T @ p_jc  (hext = [h | 1]; row 64 = softmax denom)
  - finalize: transpose back, scale by 1/denom, ELU, DMA out.
"""
import os
import sys
import numpy as np

sys.path.insert(0, "/opt/trn_rl_repo")

import concourse.bass as bass
import concourse.bacc as bacc
import concourse.tile as tile
from concourse import mybir
from concourse.masks import make_identity
from concourse import bass_utils

P = 128
N = 8192
DIN = 256
DOUT = 64
NCORES = 8
R = N // NCORES          # rows per core
BIG = 1024.0
ALPHA = 0.2
JT_W = 2048              # j supertile width (dev-tunable)
def _jt():
    return JT_W, N // JT_W, JT_W // P
NJC = N // P             # 64
NIC = N // P             # 64 chunks for h prep
SUBS = R // P            # 8 row sub-blocks per core
F32 = mybir.dt.float32
F16 = mybir.dt.float16
BF16 = mybir.dt.bfloat16
I32 = mybir.dt.int32

_cached = {}
ABLATE = set()   # dev-only: {'act','transp','srcadd','agg','dma'}
UBUFS = 2
ADJ_BUFS = 20
ACT1_SPLIT = False
AGG_DELAY = 1
PBUFS = 6
DVE_LEAKY = True
JT_FRACTION = 0


def build_module(rep=1, rep_loop=1):
    key = ("nc", rep, rep_loop)
    if key in _cached:
        return _cached[key]
    nc = bacc.Bacc("TRN2", target_bir_lowering=False, debug=False, num_devices=NCORES)

    adj_d = nc.dram_tensor("adjslab", [R, N], I32, kind="ExternalInput").ap()
    x_d = nc.dram_tensor("xt", [DIN, N], F32, kind="ExternalInput").ap()
    w_d = nc.dram_tensor("w", [DIN, DOUT], F32, kind="ExternalInput").ap()
    a_d = nc.dram_tensor("av", [2 * DOUT, 1], F32, kind="ExternalInput").ap()
    out_d = nc.dram_tensor("out", [R, DOUT], F32, kind="ExternalOutput").ap()

    with tile.TileContext(nc) as tc:
        for _ in range(rep):
            _build(nc, tc, adj_d, x_d, w_d, a_d, out_d, rep_loop)

    nc.compile()
    _cached[key] = nc
    return nc


def _build(nc, tc, adj_d, x_d, w_d, a_d, out_d, rep_loop=1):
    from contextlib import ExitStack

    with ExitStack() as ctx:
        const = ctx.enter_context(tc.tile_pool(name="const", bufs=1))

        # ---- constants ----
        bigI = const.tile([P, P], BF16)
        make_identity(nc, bigI)
        nc.vector.tensor_scalar_mul(bigI, bigI, BIG)
        idf = const.tile([P, P], F32)
        make_identity(nc, idf)
        ones2 = const.tile([2, P], BF16)
        nc.vector.memset(ones2, 1.0)
        alpha_t = const.tile([P, 1], F32)
        nc.vector.memset(alpha_t, ALPHA)

        w_a = const.tile([P, DOUT], F32)
        w_b = const.tile([P, DOUT], F32)
        nc.sync.dma_start(out=w_a, in_=w_d[0:P, :])
        nc.sync.dma_start(out=w_b, in_=w_d[P:DIN, :])
        a1 = const.tile([DOUT, 1], F32)
        a2 = const.tile([DOUT, 1], F32)
        nc.sync.dma_start(out=a1, in_=a_d[0:DOUT, :])
        nc.sync.dma_start(out=a2, in_=a_d[DOUT : 2 * DOUT, :])

        # persistent per-core data
        hext_c = [const.tile([P, 65], BF16, tag=f"hx{c}", name=f"hx{c}")
                  for c in range(NJC)]          # [j%P, f|one] per j-chunk
        dstb8 = [const.tile([P, 8], F32, tag=f"db{b}", name=f"db{b}")
                 for b in range(NJC // 8)]      # dst - BIG, batches of 8 chunks
        src_my = const.tile([1, R], F32)
        src_hi = const.tile([1, R], BF16)
        src_lo = const.tile([1, R], BF16)

        # ---- stage A: h = X@W (from pre-transposed X), hT, src, dst ----
        # Ordered for earliest stage-B unblock: chunked X loads -> hT (f32r,
        # fast) -> src -> dst -> h/hext chunks (streamed, consumed lazily by
        # the deferred aggregation matmuls).
        XCH = 4                 # x chunk tiles per half
        XW = N // XCH           # 2048 cols per chunk
        with tc.tile_pool(name="prep", bufs=1) as prep, \
             tc.tile_pool(name="prep_ps", bufs=2, space="PSUM") as prep_ps:
            xt_t = [prep.tile([P, XW], F32, tag=f"xta{k}", name=f"xta{k}")
                    for k in range(XCH)]
            xt_b = [prep.tile([P, XW], F32, tag=f"xtb{k}", name=f"xtb{k}")
                    for k in range(XCH)]
            ht_sb = prep.tile([DOUT, N], F32)   # h^T


            pid = nc.partition_id()

            for k in range(XCH):
                nc.sync.dma_start(out=xt_t[k], in_=x_d[0:P, k * XW : (k + 1) * XW])
                nc.sync.dma_start(out=xt_b[k], in_=x_d[P:DIN, k * XW : (k + 1) * XW])
            # hT [64, N] via f32r (1 cyc/col at >=256 free), dst batches
            # interleaved so dstb8[b] unblocks ACT1 as early as possible.
            for m in range(N // 512):
                k, off = m // (XW // 512), (m % (XW // 512)) * 512
                ht_ps = prep_ps.tile([DOUT, 512], F32, tag="pp", name="ht_ps")
                nc.tensor.matmul(ht_ps, lhsT=w_a,
                                 rhs=xt_t[k][:, off : off + 512],
                                 start=True, stop=False)
                nc.tensor.matmul(ht_ps, lhsT=w_b,
                                 rhs=xt_b[k][:, off : off + 512],
                                 start=False, stop=True)
                if m % 2 == 0:
                    nc.vector.tensor_copy(ht_sb[:, m * 512 : (m + 1) * 512], ht_ps)
                else:
                    nc.scalar.copy(ht_sb[:, m * 512 : (m + 1) * 512], ht_ps)
                if m % 2 == 1:
                    b = (m - 1) // 2
                    d_ps = prep_ps.tile([P, 8], F32, tag="pp", name="d_ps")
                    for bb in range(8):
                        c = b * 8 + bb
                        nc.tensor.matmul(d_ps[:, bb : bb + 1],
                                         lhsT=ht_sb[:, c * P : (c + 1) * P], rhs=a2,
                                         start=True, stop=True)
                    nc.vector.tensor_scalar_add(dstb8[b], d_ps, -BIG)
                    # h chunks -> hext tiles [128 i, 64] (+ ones col)
                    for c in range(b * 8, (b + 1) * 8):
                        kk, off2 = c // (XW // P), (c % (XW // P)) * P
                        h_ps = prep_ps.tile([P, DOUT], F32, tag="pp", name="h_ps")
                        nc.tensor.matmul(h_ps, lhsT=xt_t[kk][:, off2 : off2 + P],
                                         rhs=w_a, start=True, stop=False)
                        nc.tensor.matmul(h_ps, lhsT=xt_b[kk][:, off2 : off2 + P],
                                         rhs=w_b, start=False, stop=True)
                        if c % 2 == 0:
                            nc.scalar.copy(hext_c[c][:, 0:DOUT], h_ps)
                        else:
                            nc.vector.tensor_copy(hext_c[c][:, 0:DOUT], h_ps)
                        nc.vector.memset(hext_c[c][:, DOUT : DOUT + 1], 1.0)

            # src for this core's rows (dynamic SBUF slice by partition id)
            for ib in range(2):
                s_ps = prep_ps.tile([1, 512], F32, tag="pp", name="s_ps")
                nc.tensor.matmul(
                    s_ps, lhsT=a1,
                    rhs=ht_sb[0:DOUT, bass.ds(pid * R + ib * 512, 512)],
                    start=True, stop=True)
                nc.vector.tensor_copy(src_my[:, ib * 512 : (ib + 1) * 512], s_ps)
            nc.vector.tensor_copy(src_hi, src_my)
            nc.vector.tensor_tensor(out=src_lo, in0=src_my, in1=src_hi,
                                    op=mybir.AluOpType.subtract)

        # ---- stage B: main attention loop ----
        adjf_pool = ctx.enter_context(tc.tile_pool(name="adjf", bufs=ADJ_BUFS))
        agg_pool = ctx.enter_context(tc.tile_pool(name="agg_ps", bufs=2, space="PSUM"))
        e_pool = ctx.enter_context(tc.tile_pool(name="e_sb", bufs=4))
        p_pool = ctx.enter_context(tc.tile_pool(name="p_sb", bufs=PBUFS))

        agg = [agg_pool.tile([65, 512], F32, tag=f"agg{ib}", name=f"agg{ib}", bufs=1)
               for ib in range(2)]

        from contextlib import nullcontext
        with tc.tile_pool(name="u_ps", bufs=UBUFS, space="PSUM") as u_pool:
            loop_cm = tc.For_i(0, rep_loop, 1) if rep_loop > 1 else nullcontext()
            with loop_cm:
                _stageB(nc, tc, adj_d, adjf_pool, u_pool, e_pool, p_pool,
                        agg, bigI, ones2, (src_hi, src_lo), dstb8, alpha_t, hext_c)

        # ---- finalize ----
        with tc.tile_pool(name="fin", bufs=4) as fin, \
             tc.tile_pool(name="fin_ps", bufs=2, space="PSUM") as fin_ps:
            for ib in range(2):
                agg_sb = fin.tile([65, 512], F32, tag="agg_sb")
                nc.vector.tensor_copy(agg_sb, agg[ib])
                for q in range(4):
                    o_ps = fin_ps.tile([P, 65], F32, tag="o_ps")
                    nc.tensor.matmul(o_ps, lhsT=agg_sb[:, q * P : (q + 1) * P],
                                     rhs=idf[0:65, 0:65], start=True, stop=True)
                    rc = fin.tile([P, 1], F32, tag="rc")
                    nc.vector.reciprocal(rc, o_ps[:, DOUT : DOUT + 1])
                    hp = fin.tile([P, DOUT], F32, tag="hp")
                    nc.vector.tensor_scalar_mul(hp, o_ps[:, 0:DOUT], rc)
                    # elu = max(x,0) + exp(min(x,0)) - 1
                    ng = fin.tile([P, DOUT], F32, tag="ng")
                    nc.vector.tensor_scalar_min(ng, hp, 0.0)
                    ex = fin.tile([P, DOUT], F32, tag="ex")
                    nc.scalar.activation(ex, ng, mybir.ActivationFunctionType.Exp)
                    ps_ = fin.tile([P, DOUT], F32, tag="ps_")
                    nc.vector.tensor_scalar_max(ps_, hp, 0.0)
                    ob = fin.tile([P, DOUT], F32, tag="ob")
                    nc.vector.tensor_tensor(out=ob, in0=ex, in1=ps_,
                                            op=mybir.AluOpType.add)
                    nc.vector.tensor_scalar_add(ob, ob, -1.0)
                    g = ib * 4 + q
                    nc.sync.dma_start(out=out_d[g * P : (g + 1) * P, :], in_=ob)


def kernel(**inputs) -> np.ndarray:
    xt = np.ascontiguousarray(np.asarray(inputs["input"], np.float32)[0].T)
    adj = np.ascontiguousarray(np.asarray(inputs["adj"], np.int32))
    w = np.ascontiguousarray(np.asarray(inputs["w"], np.float32))
    a = np.ascontiguousarray(np.asarray(inputs["a"], np.float32).reshape(2 * DOUT, 1))

    nc = build_module()
    in_maps = []
    for c in range(NCORES):
        in_maps.append({
            "adjslab": adj[c * R : (c + 1) * R, :],
            "xt": xt,
            "w": w,
            "av": a,
        })
    res = bass_utils.run_bass_kernel_spmd(nc, in_maps, core_ids=list(range(NCORES)))
    out = np.concatenate([res.results[c]["out"] for c in range(NCORES)], axis=0)
    return out.astype(np.float32)


if __name__ == "__main__":
    rng = np.random.default_rng(0)
    ins = {
        "input": rng.standard_normal((1, N, DIN)).astype(np.float32),
        "adj": rng.integers(0, 2, size=(N, N)).astype(np.int32),
        "w": rng.standard_normal((DIN, DOUT)).astype(np.float32) * 0.1,
        "a": rng.standard_normal((2 * DOUT, 1)).astype(np.float32) * 0.1,
    }
    o = kernel(**ins)
    print("kernel out", o.shape, o.dtype)


def _stageB(nc, tc, adj_d, adjf_pool, u_pool, e_pool, p_pool,
            agg, bigI, ones2, src2, dstb8, alpha_t, hext_c):
        JTW, NJT, JCPJT = _jt()
        ebig = None
        pending = []

        def emit_agg(pbig_, jc_pair):
            gw = 32 if "agg" in ABLATE else 512
            for half, jcx in ((0, jc_pair), (1, jc_pair + 1)):
                for ib in range(2):
                    nc.tensor.matmul(
                        agg[ib][:, 0:gw],
                        lhsT=hext_c[jcx],
                        rhs=pbig_[:, half * 1024 + ib * 512 : half * 1024 + ib * 512 + gw],
                        start=(jcx == 0),
                        stop=(jcx == NJC - 1),
                    )

        for jt in range(NJT if not JT_FRACTION else max(1, NJT // JT_FRACTION)):
            adjf = []
            for s in range(SUBS):
                t = adjf_pool.tile([P, JTW], BF16, tag="adjf")
                if "dma" in ABLATE:
                    if s == 0:
                        nc.gpsimd.dma_start(out=t[:, 0:32],
                            in_=adj_d[s * P : (s + 1) * P, jt * JTW : jt * JTW + 32])
                    else:
                        nc.vector.memset(t[:, 0:32], 0.0)
                else:
                    nc.gpsimd.dma_start(
                        out=t,
                        in_=adj_d[s * P : (s + 1) * P, jt * JTW : (jt + 1) * JTW],
                    )
                adjf.append(t)
            for k in range(JCPJT):
                jc = jt * JCPJT + k
                u = u_pool.tile([P, 1024], F32, tag="u")
                tw = 16 if "transp" in ABLATE else P
                for q in range(8):
                    ib, qq = q // 4, q % 4
                    nc.tensor.matmul(
                        u[:, ib * 512 + qq * P : ib * 512 + qq * P + tw],
                        lhsT=adjf[q][:, k * P : (k + 1) * P],
                        rhs=bigI[:, 0:tw],
                        start=(qq == 0),
                        stop=False,
                    )
                sw = 16 if "srcadd" in ABLATE else 512
                for ib in range(2):
                    nc.tensor.matmul(
                        u[:, ib * 512 : ib * 512 + sw],
                        lhsT=ones2[0:1, :],
                        rhs=src2[0][:, ib * 512 : ib * 512 + sw],
                        start=False,
                        stop=False,
                    )
                    nc.tensor.matmul(
                        u[:, ib * 512 : ib * 512 + sw],
                        lhsT=ones2[0:1, :],
                        rhs=src2[1][:, ib * 512 : ib * 512 + sw],
                        start=False,
                        stop=True,
                    )
                if jc % 2 == 0:
                    ebig = e_pool.tile([P, 2048], F32, tag="ebig")
                aw = 32 if "act" in ABLATE else 1024
                if ACT1_SPLIT and aw == 1024:
                    for hb in range(2):
                        nc.scalar.activation(
                            ebig[:, (jc % 2) * 1024 + hb * 512 : (jc % 2) * 1024 + (hb + 1) * 512],
                            u[:, hb * 512 : (hb + 1) * 512],
                            mybir.ActivationFunctionType.Prelu,
                            bias=dstb8[jc // 8][:, jc % 8 : jc % 8 + 1],
                            scale=1.0,
                            alpha=alpha_t,
                        )
                elif DVE_LEAKY and jc % 2 == 1:
                    eb = ebig[:, 1024 : 1024 + aw]
                    tmp = e_pool.tile([P, 1024], F32, tag="lk", name="lk")
                    nc.vector.tensor_scalar(
                        out=eb, in0=u[:, 0:aw],
                        scalar1=dstb8[jc // 8][:, jc % 8 : jc % 8 + 1],
                        scalar2=None, op0=mybir.AluOpType.add)
                    nc.vector.tensor_scalar_mul(tmp[:, 0:aw], eb, 0.2)
                    nc.vector.tensor_tensor(out=eb, in0=eb, in1=tmp[:, 0:aw],
                                            op=mybir.AluOpType.max)
                else:
                    nc.scalar.activation(
                        ebig[:, (jc % 2) * 1024 : (jc % 2) * 1024 + aw],
                        u[:, 0:aw],
                        mybir.ActivationFunctionType.Prelu,
                        bias=dstb8[jc // 8][:, jc % 8 : jc % 8 + 1],
                        scale=1.0,
                        alpha=alpha_t,
                    )
                if jc % 2 == 1:
                    pbig = p_pool.tile([P, 2048], BF16, tag="pbig")
                    pw = 32 if "act" in ABLATE else 2048
                    nc.scalar.activation(
                        pbig[:, 0:pw], ebig[:, 0:pw], mybir.ActivationFunctionType.Exp
                    )
                    pending.append((pbig, jc - 1))
                    if len(pending) > AGG_DELAY:
                        emit_agg(*pending.pop(0))
        while pending:
            emit_agg(*pending.pop(0))



# revision 7
# speedup vs baseline: 1.6119x; 1.6119x over previous
"""GAT head (DGAT) Trainium2 kernel: 8-core row-sharded masked-softmax attention.

Math (per reference):
  h = X @ W                       [N, 64]
  e = leaky_relu(src_i + dst_j, 0.2), src = h@a[:64], dst = h@a[64:]
  att = softmax(where(adj>0, e, -9e15), axis=1)
  out = elu(att @ h)

Key identity: softmax rows are invariant to per-row scaling, and
  exp(leaky_relu(x)) = max(e^x, e^{0.2x}).
Scaling row i by e^{-src_i} and factoring the per-j part into the
aggregation weights gives, with x = src_i + dst_j:
  p~_ij = adj_ji * max(e^{dst_j}, e^{0.2 dst_j - 0.8 src_i})
        = B_j * adj_ji * max(1, A2_i * D_j)
  where A2_i = e^{-0.8 src_i}, D_j = e^{-0.8 dst_j}, B_j = e^{dst_j}.
B_j folds into the aggregation matrix hextB = [h*B | B]; the leftover
q = max(u1, u2*D_j) needs only two diag-scaled PE transposes of adj
(u1 = adjT, u2 = adjT*A2_i) and one fused DVE scalar_tensor_tensor.

Per core c (rows R = [1024c, 1024c+1024)):
  - adj shipped from host as fp8 (0/1), raw hw-DGE DMA (1B/elem, no cast)
  - PE: u1[j,i] = adjT (fp8 x bf16-identity), ACT: evacuate u1 -> SBUF bf16
  - PE: u2[j,i] = adjT*A2_i (fp8 x bf16-diag)
  - DVE: q = max(u2*D_j, u1)  (fused STT, psum+sbuf -> bf16)
  - PE: agg[65, i] += hextB_jc^T @ q  (row 64 = softmax denominator)
  - finalize: transpose back, divide, ELU, DMA out.
"""
import os
import sys
import numpy as np

sys.path.insert(0, "/opt/trn_rl_repo")

import concourse.bass as bass
import concourse.bacc as bacc
import concourse.tile as tile
from concourse import mybir
from concourse.masks import make_identity
from concourse import bass_utils

P = 128
N = 8192
DIN = 256
DOUT = 64
NCORES = 8
R = N // NCORES          # rows per core
JT_W = 2048              # j supertile width
NJT = N // JT_W          # 4
JCPJT = JT_W // P        # 16
NJC = N // P             # 64
SUBS = R // P            # 8 row sub-blocks per core
F32 = mybir.dt.float32
BF16 = mybir.dt.bfloat16
F8 = mybir.dt.float8e4

_cached = {}
ABLATE = set()           # dev-only: {'dma','transp','evac','stt','agg'}
UBUFS = 3
ADJ_BUFS = 20
SB1_BUFS = 4
AGG_DELAY = 2
PBUFS = 6


def build_module(rep=1, rep_loop=1):
    key = ("nc", rep, rep_loop)
    if key in _cached:
        return _cached[key]
    nc = bacc.Bacc("TRN2", target_bir_lowering=False, debug=False, num_devices=NCORES)

    adj_d = nc.dram_tensor("adjslab", [R, N], F8, kind="ExternalInput").ap()
    x_d = nc.dram_tensor("xt", [DIN, N], F32, kind="ExternalInput").ap()
    w_d = nc.dram_tensor("w", [DIN, DOUT], F32, kind="ExternalInput").ap()
    a_d = nc.dram_tensor("av", [2 * DOUT, 1], F32, kind="ExternalInput").ap()
    out_d = nc.dram_tensor("out", [R, DOUT], F32, kind="ExternalOutput").ap()

    with tile.TileContext(nc) as tc:
        for _ in range(rep):
            _build(nc, tc, adj_d, x_d, w_d, a_d, out_d, rep_loop)

    nc.compile()
    _cached[key] = nc
    return nc


def _build(nc, tc, adj_d, x_d, w_d, a_d, out_d, rep_loop=1):
    from contextlib import ExitStack

    with ExitStack() as ctx:
        const = ctx.enter_context(tc.tile_pool(name="const", bufs=1))

        # ---- constants ----
        id_bf = const.tile([P, P], BF16)
        make_identity(nc, id_bf)
        idf = const.tile([P, P], F32)
        make_identity(nc, idf)
        ones2 = const.tile([2, P], BF16)
        nc.vector.memset(ones2, 1.0)

        w_a = const.tile([P, DOUT], F32)
        w_b = const.tile([P, DOUT], F32)
        nc.sync.dma_start(out=w_a, in_=w_d[0:P, :])
        nc.sync.dma_start(out=w_b, in_=w_d[P:DIN, :])
        a1 = const.tile([DOUT, 1], F32)
        a2 = const.tile([DOUT, 1], F32)
        nc.sync.dma_start(out=a1, in_=a_d[0:DOUT, :])
        nc.sync.dma_start(out=a2, in_=a_d[DOUT : 2 * DOUT, :])

        # persistent per-core data
        hext_c = [const.tile([P, 65], BF16, tag=f"hx{c}", name=f"hx{c}")
                  for c in range(NJC)]          # [j%P, h*B | B] per j-chunk
        dstD8 = [const.tile([P, 8], F32, tag=f"dD{b}", name=f"dD{b}")
                 for b in range(NJC // 8)]      # e^{-0.8 dst}, batches of 8 chunks
        diagA2 = [const.tile([P, P], BF16, tag=f"dg{s}", name=f"dg{s}")
                  for s in range(SUBS)]         # diag(e^{-0.8 src}) per row block
        src_my = const.tile([1, R], F32)

        # ---- stage A: h = X@W (from pre-transposed X), hT, src, dst ----
        XCH = 4                 # x chunk tiles per half
        XW = N // XCH           # 2048 cols per chunk
        with tc.tile_pool(name="prep", bufs=1) as prep, \
             tc.tile_pool(name="prep_ps", bufs=2, space="PSUM") as prep_ps:
            xt_t = [prep.tile([P, XW], F32, tag=f"xta{k}", name=f"xta{k}")
                    for k in range(XCH)]
            xt_b = [prep.tile([P, XW], F32, tag=f"xtb{k}", name=f"xtb{k}")
                    for k in range(XCH)]
            ht_sb = prep.tile([DOUT, N], F32)   # h^T
            dstB8 = [prep.tile([P, 8], F32, tag=f"dB{b}", name=f"dB{b}")
                     for b in range(NJC // 8)]  # e^{dst}

            pid = nc.partition_id()

            for k in range(XCH):
                nc.sync.dma_start(out=xt_t[k], in_=x_d[0:P, k * XW : (k + 1) * XW])
                nc.sync.dma_start(out=xt_b[k], in_=x_d[P:DIN, k * XW : (k + 1) * XW])
            for m in range(N // 512):
                k, off = m // (XW // 512), (m % (XW // 512)) * 512
                ht_ps = prep_ps.tile([DOUT, 512], F32, tag="pp", name="ht_ps")
                nc.tensor.matmul(ht_ps, lhsT=w_a,
                                 rhs=xt_t[k][:, off : off + 512],
                                 start=True, stop=False)
                nc.tensor.matmul(ht_ps, lhsT=w_b,
                                 rhs=xt_b[k][:, off : off + 512],
                                 start=False, stop=True)
                if m % 2 == 0:
                    nc.vector.tensor_copy(ht_sb[:, m * 512 : (m + 1) * 512], ht_ps)
                else:
                    nc.scalar.copy(ht_sb[:, m * 512 : (m + 1) * 512], ht_ps)
                if m % 2 == 1:
                    b = (m - 1) // 2
                    d_ps = prep_ps.tile([P, 8], F32, tag="pp", name="d_ps")
                    for bb in range(8):
                        c = b * 8 + bb
                        nc.tensor.matmul(d_ps[:, bb : bb + 1],
                                         lhsT=ht_sb[:, c * P : (c + 1) * P], rhs=a2,
                                         start=True, stop=True)
                    nc.scalar.activation(dstD8[b], d_ps,
                                         mybir.ActivationFunctionType.Exp,
                                         scale=-0.8)
                    nc.scalar.activation(dstB8[b], d_ps,
                                         mybir.ActivationFunctionType.Exp,
                                         scale=1.0)
                    # h chunks -> hextB tiles [128 j, 64 f]*B (+ B col)
                    for c in range(b * 8, (b + 1) * 8):
                        kk, off2 = c // (XW // P), (c % (XW // P)) * P
                        h_ps = prep_ps.tile([P, DOUT], F32, tag="pp", name="h_ps")
                        nc.tensor.matmul(h_ps, lhsT=xt_t[kk][:, off2 : off2 + P],
                                         rhs=w_a, start=True, stop=False)
                        nc.tensor.matmul(h_ps, lhsT=xt_b[kk][:, off2 : off2 + P],
                                         rhs=w_b, start=False, stop=True)
                        nc.vector.tensor_scalar_mul(
                            hext_c[c][:, 0:DOUT], h_ps, dstB8[b][:, c % 8 : c % 8 + 1])
                        nc.vector.tensor_copy(
                            hext_c[c][:, DOUT : DOUT + 1],
                            dstB8[b][:, c % 8 : c % 8 + 1])

            # src for this core's rows (dynamic SBUF slice by partition id)
            for ib in range(2):
                s_ps = prep_ps.tile([1, 512], F32, tag="pp", name="s_ps")
                nc.tensor.matmul(
                    s_ps, lhsT=a1,
                    rhs=ht_sb[0:DOUT, bass.ds(pid * R + ib * 512, 512)],
                    start=True, stop=True)
                nc.vector.tensor_copy(src_my[:, ib * 512 : (ib + 1) * 512], s_ps)
            # diag(e^{-0.8 src}) blocks
            a2row = prep.tile([1, R], F32)
            nc.scalar.activation(a2row, src_my,
                                 mybir.ActivationFunctionType.Exp, scale=-0.8)
            a2row_bf = prep.tile([1, R], BF16)
            nc.vector.tensor_copy(a2row_bf, a2row)
            for s in range(SUBS):
                mm_ps = prep_ps.tile([P, P], F32, tag="pp", name="mm_ps")
                nc.tensor.matmul(mm_ps, lhsT=ones2[0:1, :],
                                 rhs=a2row_bf[:, s * P : (s + 1) * P],
                                 start=True, stop=True)
                nc.vector.tensor_tensor(out=diagA2[s], in0=id_bf, in1=mm_ps,
                                        op=mybir.AluOpType.mult)

        # ---- stage B: main attention loop ----
        adjf_pool = ctx.enter_context(tc.tile_pool(name="adjf", bufs=ADJ_BUFS))
        agg_pool = ctx.enter_context(tc.tile_pool(name="agg_ps", bufs=2, space="PSUM"))
        sb1_pool = ctx.enter_context(tc.tile_pool(name="u1sb", bufs=SB1_BUFS))
        p_pool = ctx.enter_context(tc.tile_pool(name="q_sb", bufs=PBUFS))

        agg = [agg_pool.tile([65, 512], F32, tag=f"agg{ib}", name=f"agg{ib}", bufs=1)
               for ib in range(2)]

        from contextlib import nullcontext
        with tc.tile_pool(name="u_ps", bufs=UBUFS, space="PSUM") as u_pool:
            loop_cm = tc.For_i(0, rep_loop, 1) if rep_loop > 1 else nullcontext()
            with loop_cm:
                _stageB(nc, tc, adj_d, adjf_pool, u_pool, sb1_pool, p_pool,
                        agg, id_bf, diagA2, dstD8, hext_c)

        # ---- finalize ----
        with tc.tile_pool(name="fin", bufs=4) as fin, \
             tc.tile_pool(name="fin_ps", bufs=2, space="PSUM") as fin_ps:
            for ib in range(2):
                agg_sb = fin.tile([65, 512], F32, tag="agg_sb")
                nc.vector.tensor_copy(agg_sb, agg[ib])
                for q in range(4):
                    o_ps = fin_ps.tile([P, 65], F32, tag="o_ps")
                    nc.tensor.matmul(o_ps, lhsT=agg_sb[:, q * P : (q + 1) * P],
                                     rhs=idf[0:65, 0:65], start=True, stop=True)
                    rc = fin.tile([P, 1], F32, tag="rc")
                    nc.vector.reciprocal(rc, o_ps[:, DOUT : DOUT + 1])
                    hp = fin.tile([P, DOUT], F32, tag="hp")
                    nc.vector.tensor_scalar_mul(hp, o_ps[:, 0:DOUT], rc)
                    # elu = max(x,0) + exp(min(x,0)) - 1
                    ng = fin.tile([P, DOUT], F32, tag="ng")
                    nc.vector.tensor_scalar_min(ng, hp, 0.0)
                    ex = fin.tile([P, DOUT], F32, tag="ex")
                    nc.scalar.activation(ex, ng, mybir.ActivationFunctionType.Exp)
                    ps_ = fin.tile([P, DOUT], F32, tag="ps_")
                    nc.vector.tensor_scalar_max(ps_, hp, 0.0)
                    ob = fin.tile([P, DOUT], F32, tag="ob")
                    nc.vector.tensor_tensor(out=ob, in0=ex, in1=ps_,
                                            op=mybir.AluOpType.add)
                    nc.vector.tensor_scalar_add(ob, ob, -1.0)
                    g = ib * 4 + q
                    nc.sync.dma_start(out=out_d[g * P : (g + 1) * P, :], in_=ob)


def _stageB(nc, tc, adj_d, adjf_pool, u_pool, sb1_pool, p_pool,
            agg, id_bf, diagA2, dstD8, hext_c):
    pending = []

    def emit_agg(q_t, jc):
        gw = 32 if "agg" in ABLATE else 512
        for ib in range(2):
            nc.tensor.matmul(
                agg[ib][:, 0:gw],
                lhsT=hext_c[jc],
                rhs=q_t[:, ib * 512 : ib * 512 + gw],
                start=(jc == 0),
                stop=(jc == NJC - 1),
            )

    for jt in range(NJT):
        adjf = []
        for s in range(SUBS):
            t = adjf_pool.tile([P, JT_W], F8, tag="adjf")
            if "dma" in ABLATE:
                nc.vector.memset(t, 1.0)
            else:
                nc.sync.dma_start(
                    out=t,
                    in_=adj_d[s * P : (s + 1) * P, jt * JT_W : (jt + 1) * JT_W],
                )
            adjf.append(t)
        for k in range(JCPJT):
            jc = jt * JCPJT + k
            tw = 16 if "transp" in ABLATE else P
            u1 = u_pool.tile([P, 1024], F32, tag="u")
            for q8 in range(8):
                ib, qq = q8 // 4, q8 % 4
                nc.tensor.matmul(
                    u1[:, ib * 512 + qq * P : ib * 512 + qq * P + tw],
                    lhsT=adjf[q8][:, k * P : (k + 1) * P],
                    rhs=id_bf[:, 0:tw],
                    start=(qq == 0),
                    stop=(qq == 3),
                )
            u1sb = sb1_pool.tile([P, 1024], BF16, tag="u1sb")
            if "evac" in ABLATE:
                nc.scalar.copy(u1sb[:, 0:32], u1[:, 0:32])
            else:
                nc.scalar.copy(u1sb, u1)
            u2 = u_pool.tile([P, 1024], F32, tag="u")
            for q8 in range(8):
                ib, qq = q8 // 4, q8 % 4
                nc.tensor.matmul(
                    u2[:, ib * 512 + qq * P : ib * 512 + qq * P + tw],
                    lhsT=adjf[q8][:, k * P : (k + 1) * P],
                    rhs=diagA2[q8][:, 0:tw],
                    start=(qq == 0),
                    stop=(qq == 3),
                )
            qt = p_pool.tile([P, 1024], BF16, tag="q")
            sttw = 32 if "stt" in ABLATE else 1024
            nc.vector.scalar_tensor_tensor(
                out=qt[:, 0:sttw], in0=u2[:, 0:sttw],
                scalar=dstD8[jc // 8][:, jc % 8 : jc % 8 + 1],
                in1=u1sb[:, 0:sttw],
                op0=mybir.AluOpType.mult, op1=mybir.AluOpType.max)
            pending.append((qt, jc))
            if len(pending) > AGG_DELAY:
                emit_agg(*pending.pop(0))
    while pending:
        emit_agg(*pending.pop(0))


def kernel(**inputs) -> np.ndarray:
    import ml_dtypes
    xt = np.ascontiguousarray(np.asarray(inputs["input"], np.float32)[0].T)
    adj = np.asarray(inputs["adj"])
    adj_f8 = (adj > 0).astype(ml_dtypes.float8_e4m3fn)
    w = np.ascontiguousarray(np.asarray(inputs["w"], np.float32))
    a = np.ascontiguousarray(np.asarray(inputs["a"], np.float32).reshape(2 * DOUT, 1))

    nc = build_module()
    in_maps = []
    for c in range(NCORES):
        in_maps.append({
            "adjslab": np.ascontiguousarray(adj_f8[c * R : (c + 1) * R, :]),
            "xt": xt,
            "w": w,
            "av": a,
        })
    res = bass_utils.run_bass_kernel_spmd(nc, in_maps, core_ids=list(range(NCORES)))
    out = np.concatenate([res.results[c]["out"] for c in range(NCORES)], axis=0)
    return out.astype(np.float32)


def make_in_maps(inputs):
    """Prepped per-core input maps (host-side reformat, shared with test.py)."""
    import ml_dtypes
    xt = np.ascontiguousarray(np.asarray(inputs["input"], np.float32)[0].T)
    adj_f8 = (np.asarray(inputs["adj"]) > 0).astype(ml_dtypes.float8_e4m3fn)
    return [{
        "adjslab": np.ascontiguousarray(adj_f8[c * R : (c + 1) * R, :]),
        "xt": xt,
        "w": np.asarray(inputs["w"], np.float32),
        "av": np.asarray(inputs["a"], np.float32).reshape(-1, 1),
    } for c in range(NCORES)]


if __name__ == "__main__":
    rng = np.random.default_rng(0)
    ins = {
        "input": rng.standard_normal((1, N, DIN)).astype(np.float32),
        "adj": rng.integers(0, 2, size=(N, N)).astype(np.int32),
        "w": rng.standard_normal((DIN, DOUT)).astype(np.float32) * 0.1,
        "a": rng.standard_normal((2 * DOUT, 1)).astype(np.float32) * 0.1,
    }
    o = kernel(**ins)
    print("kernel out", o.shape, o.dtype)
